# revision 18
# baseline (speedup 1.0000x reference)
"""Sparse-attention head kernel for Trainium2, data-parallel over batch on 8 cores.

Math per batch b (see reference):
  q,k,v = x @ W{q,k,v}.T + b{q,k,v}          # [T, 64]
  qg    = q[keep]                            # [K=T/2, 64]
  att   = softmax(mask(qg @ k.T / sqrt(C)))  # [K, T], row i allows t <= keep[i]
  out   = att @ v                            # [K, 64]

Device strategy (per core, one batch):
  - x[b].T uploaded bf16; first 3 t-chunks via SWDGE (Pool engine starts
    ~0.2us, HWDGE queues need ~7us to boot), rest via HWDGE, one 3D DMA
    per 512-column chunk
  - projections: per 128-t-block, qkv[t, q|k|v|1] = sum_c xT_c.T @ W_c
    packed into one persistent [128, 32*193] tile; one DVE bias-add per
    block; PSUM holds two blocks per bank to deepen the pipeline
  - kT stored two-deep (even t-blocks on partitions 0-63, odd on 64-127)
    so score matmuls for a t-block pair run as two concurrent row-group
    tiles (K=64 each) on the PE array -> ~2x ST throughput; kT copies run
    on the scalar engine, which is idle until the first exp anyway
  - qgT duplicated onto both partition halves (two PE transposes into one
    PSUM tile) to feed both row groups
  - q rows for the first two q-chunks round-trip DRAM; gathers fire per
    128-row j-block as soon as the needed t-prefix is flushed, so the
    first score matmul (and exp) start ~20us in.  The last two q-chunks
    have contiguous keep rows (t 3072..4095) and transpose straight out
    of the projection tiles
  - boundary masks applied additively (-1e30) into the score PSUM by the
    PE itself (identity-stationary accumulate), so exp depends only on PE
  - transposed attention: S_T[t,q] in PSUM, E = exp(S/sqrt(C)) on ACT,
    out_T[65,q] = sum_t [v|1].T @ E  (row 64 = softmax denominator)
  - PE-transpose out_T, divide by denominator, DMA out
All matmul inputs bf16 (fp32 accumulation in PSUM); final epilogue in fp32.
"""

import math
import os

if "JAX_PLATFORMS" not in os.environ:
    os.environ["JAX_PLATFORMS"] = "axon,cpu"

import numpy as np
import ml_dtypes

B, T, C = 8, 4096, 1024
HS = 64
KQ = T // 2  # 2048 gathered query rows
NCORES = 8
SCALE = float(C) ** -0.5
QC = 512   # attention q-chunk (matmul moving width)
BF16 = ml_dtypes.bfloat16
NQC = KQ // QC  # 4
NEG = -1.0e30
BW = HS * 3 + 1  # 193: q|k|v|1 per t-block


def _keep_indices(t):
    a = math.ceil(t / 4)
    keep = [t - 1 - x for x in range(a)]
    keep += [t - 1 - math.ceil(3 / a * (x - a) ** 2 + a) for x in range(a, math.ceil(t / 2))]
    return np.array(list(reversed(keep)), dtype=np.int64)


KEEP = _keep_indices(T)  # [KQ], ascending
# last KQ/2 keep rows are exactly t = T-KQ/2 .. T-1 (contiguous)
assert (KEEP[KQ // 2:] == np.arange(T - KQ // 2, T)).all()

# Static block classification at [t=128] x [q=128] granularity.
_NT = T // 128   # 32
_NJ = KQ // 128  # 16
_FULL, _BOUND, _DEAD = 0, 1, 2
_BLOCK_KIND = np.empty((_NT, _NJ), dtype=np.int64)
_MASK_IDX = {}
for _tb in range(_NT):
    for _j in range(_NJ):
        qlo = KEEP[_j * 128]
        qhi = KEEP[_j * 128 + 127]
        if 128 * _tb + 127 <= qlo:
            _BLOCK_KIND[_tb, _j] = _FULL
        elif 128 * _tb > qhi:
            _BLOCK_KIND[_tb, _j] = _DEAD
        else:
            _BLOCK_KIND[_tb, _j] = _BOUND
            _MASK_IDX[(_tb, _j)] = len(_MASK_IDX)
_NMASK = len(_MASK_IDX)

# t-blocks needed per q-chunk (all even -> full t-block pairs)
_NTB_QC = [int(KEEP[qc * QC + QC - 1]) // 128 + 1 for qc in range(NQC)]
assert all(n % 2 == 0 for n in _NTB_QC)

# q-scratch flush segments (t-block ranges) and the gather j-blocks each
# flush unlocks: gather for j needs q rows up to KEEP[j*128+127].
_FLUSH_SEGS = [(0, 6), (6, 11), (11, 15), (15, 18), (18, 24)]
_GATHER_AT = {}  # end-tb -> list of j
for _j in range(_NJ // 2):
    need = int(KEEP[_j * 128 + 127]) // 128 + 1
    for (_a, _b) in _FLUSH_SEGS:
        if _b >= need:
            _GATHER_AT.setdefault(_b, []).append(_j)
            break
assert sum(len(v) for v in _GATHER_AT.values()) == _NJ // 2
# gather DMAs have multi-us latency; transpose their results a few t-blocks
# later so the in-order PE queue never stalls waiting on a gather
_TRANSPOSE_AT = {9: [0], 13: [1], 17: [2], 21: [3], 24: [4, 5], 25: [6, 7]}


def _alive_j0(qc, tb):
    # sub-blocks j in [4qc, 4qc+4); dead ones form a prefix (keep ascending)
    for jj in range(QC // 128):
        if _BLOCK_KIND[tb, qc * (QC // 128) + jj] != _DEAD:
            return jj
    return QC // 128


def _host_masks():
    # additive masks: 0 where allowed, NEG where disallowed
    m = np.zeros((128, _NMASK * 128), dtype=np.float32)
    for (tb, j), idx in _MASK_IDX.items():
        tvals = 128 * tb + np.arange(128)[:, None]
        kvals = KEEP[j * 128:(j + 1) * 128][None, :]
        m[:, idx * 128:(idx + 1) * 128] = np.where(tvals <= kvals, 0.0, NEG)
    return m.astype(BF16)


_prog_cache = {}
TRACE = False          # set by test harness to collect an NTFF profile
TRACE_KW = {}
LAST_RESULTS = None    # BassKernelResults of the most recent kernel() call


def _build_program(reps=1):
    import concourse.bass as bass
    import concourse.mybir as mybir
    import concourse.tile as tile
    from concourse import bacc
    from concourse.masks import make_identity

    dt = mybir.dt
    f32, bf16, u32 = dt.float32, dt.bfloat16, dt.uint32
    Alu = mybir.AluOpType
    Act = mybir.ActivationFunctionType

    nc = bacc.Bacc("TRN2", target_bir_lowering=False, debug=False,
                   enable_partition_id=False)

    xt_d = nc.dram_tensor("xt", [C, T], bf16, kind="ExternalInput").ap()
    wpack_d = nc.dram_tensor("wpack", [128, 8 * 192], f32, kind="ExternalInput").ap()
    bias_d = nc.dram_tensor("bias", [1, 192], f32, kind="ExternalInput").ap()
    masks_d = nc.dram_tensor("masks", [128, _NMASK * 128], bf16, kind="ExternalInput").ap()
    keep_d = nc.dram_tensor("keepidx", [128, _NJ], u32, kind="ExternalInput").ap()
    out_d = nc.dram_tensor("out", [KQ, HS], f32, kind="ExternalOutput").ap()

    NTC = 8        # xt DMA t-chunks
    TCW = T // NTC  # 512
    NSW = 3        # leading chunks via SWDGE (Pool), rest via HWDGE (sync)
    NPAIR = _NT // 2

    with tile.TileContext(nc) as tc:
        with (
            tc.tile_pool(name="const", bufs=1) as constp,
            tc.tile_pool(name="xt", bufs=1) as xtp,
            tc.tile_pool(name="proj", bufs=1) as projp,
            tc.tile_pool(name="dram", bufs=1, space="DRAM") as dramp,
            tc.tile_pool(name="psA", bufs=2, space="PSUM") as psA,
            tc.tile_pool(name="psB", bufs=1, space="PSUM") as psB,
            tc.tile_pool(name="psS", bufs=2, space="PSUM") as psS,
            tc.tile_pool(name="psO", bufs=1, space="PSUM") as psO,
            tc.tile_pool(name="work", bufs=2) as workp,
            tc.tile_pool(name="ework", bufs=4) as ep,
        ):
            xt_big = xtp.tile([128, 8 * T], bf16, name="xt_big", tag="xt_big")
            xt3 = xt_big.rearrange("p (c t) -> p c t", c=8)
            xt_d3 = xt_d.rearrange("(c p) t -> p c t", p=128)

            def xchunk(tci, engine):
                lo, hi = tci * TCW, (tci + 1) * TCW
                engine.dma_start(out=xt3[:, :, lo:hi], in_=xt_d3[:, :, lo:hi])

            ident_b = constp.tile([128, 128], bf16)
            make_identity(nc, ident_b)
            ident_f = constp.tile([128, 128], f32)
            make_identity(nc, ident_f)
            # touch Exp so the ACT table set loads during the idle head, not
            # at the first real exp
            warm = constp.tile([128, 1], bf16)
            nc.scalar.activation(warm, ident_b[:, 0:1], Act.Exp)

            # x chunks all on one HWDGE ring, strictly ordered (mixing SWDGE
            # and HWDGE interleaves transfers on the shared DMA engines and
            # delays the first chunk's completion by several us)
            xchunk(0, nc.sync)

            wpack_sb = constp.tile([128, 8 * 192], bf16)
            nc.gpsimd.dma_start(out=wpack_sb, in_=wpack_d)
            w_sb = [wpack_sb[:, c * 192:(c + 1) * 192] for c in range(8)]
            bias_bc = constp.tile([128, 192], bf16)
            nc.gpsimd.dma_start(out=bias_bc, in_=bias_d.to_broadcast([128, 192]))

            mask_big = constp.tile([128, _NMASK * 128], bf16)
            nc.gpsimd.dma_start(out=mask_big, in_=masks_d)
            keep_big = constp.tile([128, _NJ], u32)
            nc.gpsimd.dma_start(out=keep_big, in_=keep_d)

            # ---- per-repetition kernel body (reps>1 only for timing) ----
            def emit_once(first):
                kt2 = projp.tile([128, NPAIR * 128], bf16, name="kt2", tag="kt2")
                qgt2 = projp.tile([128, KQ], bf16, name="qgt2", tag="qgt2")
                qkv = projp.tile([128, _NT * BW], bf16, name="qkv", tag="qkv")
                qkv3 = qkv.rearrange("p (b z) -> p b z", b=_NT)
                qscr = dramp.tile([T, HS], bf16, name="qscr", tag="qscr")

                # ones column of every [v|1] block, one strided memset
                nc.gpsimd.memset(qkv3[:, :, BW - 1:BW], 1.0)

                wave_state = {}

                gathered_qg = {}

                def emit_gather_dma(j):
                    # indirect row gather (j-blocks 0..7: scattered keep rows)
                    rows = 128 * (int(KEEP[j * 128 + 127]) // 128 + 1)
                    qg_g = workp.tile([128, HS], bf16, name="qg_g",
                                      tag=f"qg{j % 4}", bufs=1)
                    nc.gpsimd.indirect_dma_start(
                        out=qg_g, out_offset=None, in_=qscr[0:rows, :],
                        in_offset=bass.IndirectOffsetOnAxis(
                            ap=keep_big[:, j:j + 1], axis=0),
                    )
                    gathered_qg[j] = qg_g

                def emit_gather_transpose(j):
                    qg_g = gathered_qg.pop(j)
                    ps_q = psA.tile([128, 128], bf16, name="ps_q", tag="ps_qkv")
                    nc.tensor.transpose(ps_q[0:64, :], qg_g, ident_b)
                    nc.tensor.transpose(ps_q[64:128, :], qg_g, ident_b,
                                        tile_position=(0, 64))
                    nc.vector.tensor_copy(qgt2[:, j * 128:(j + 1) * 128], ps_q)

                def emit_pair(qc, tba, tbb):
                    """Row-tiled ST pair + PE masks + one exp; emits previous
                    pair's PVs between the STs and the exp."""
                    st = wave_state[qc]
                    if st["ps_o"] is None:
                        st["ps_o"] = psO.tile([HS + 1, QC], f32, name=f"ps_o_{qc}",
                                              tag="ps_o")
                    q0 = qc * QC
                    i = tba // 2
                    a0a = _alive_j0(qc, tba) * 128
                    a0b = _alive_j0(qc, tbb) * 128
                    ps_s = psS.tile([128, 2 * QC], f32, name="ps_s")
                    # 4 concurrent PE tiles: row groups = t-block pair (K=64
                    # each), column groups = t-offset halves (M=64 each)
                    for rg, seg, a0_ in ((0, 0, a0a), (64, QC, a0b)):
                        for ch in (0, 64):
                            nc.tensor.matmul(
                                ps_s[ch:ch + 64, seg + a0_:seg + QC],
                                lhsT=kt2[rg:rg + 64,
                                         i * 128 + ch:i * 128 + ch + 64],
                                rhs=qgt2[rg:rg + 64, q0 + a0_:q0 + QC],
                                start=True, stop=True,
                            )
                    # additive -1e30 masks for boundary blocks, accumulated by
                    # the PE itself (identity stationary) so exp waits only on PE
                    for i2, tb in enumerate((tba, tbb)):
                        for jj in range(QC // 128):
                            j = q0 // 128 + jj
                            if _BLOCK_KIND[tb, j] == _BOUND:
                                midx = _MASK_IDX[(tb, j)]
                                o = i2 * QC + jj * 128
                                nc.tensor.matmul(
                                    ps_s[:, o:o + 128], lhsT=ident_b,
                                    rhs=mask_big[:, midx * 128:(midx + 1) * 128],
                                    start=False, stop=True, skip_group_check=True,
                                )
                    prev_pv = st["pv_pending"]
                    st["pv_pending"] = None
                    if prev_pv is not None:
                        emit_pv(qc, *prev_pv)
                    e_sb = ep.tile([128, 2 * QC], bf16, name="e_sb")
                    nc.scalar.activation(e_sb[:, a0a:2 * QC], ps_s[:, a0a:2 * QC],
                                         Act.Exp, scale=SCALE)
                    st["pv_pending"] = ((tba, tbb), e_sb, (a0a, a0b))

                def emit_pv(qc, tbs, e_sb, a0s):
                    st = wave_state[qc]
                    ntb = _NTB_QC[qc]
                    for i, tb in enumerate(tbs):
                        nc.tensor.matmul(
                            st["ps_o"][:, a0s[i]:QC],
                            lhsT=qkv3[:, tb, 2 * HS:BW],
                            rhs=e_sb[:, i * QC + a0s[i]:(i + 1) * QC],
                            start=(tb == 0), stop=(tb == ntb - 1),
                        )

                def emit_epilogue(qc):
                    st = wave_state[qc]
                    if st["pv_pending"] is not None:
                        emit_pv(qc, *st["pv_pending"])
                        st["pv_pending"] = None
                    q0 = qc * QC
                    ps_o = st["ps_o"]
                    ot_sb = workp.tile([HS + 1, QC], f32, name="ot_sb", tag="ot")
                    nc.vector.tensor_copy(ot_sb, ps_o)
                    out4 = workp.tile([128, (QC // 128) * HS], f32,
                                      name="out4", tag="out4")
                    for jj in range(QC // 128):
                        ps_on = psA.tile([128, HS + 1], f32, name="ps_on",
                                         tag="ps_qkv")
                        nc.tensor.transpose(
                            ps_on, ot_sb[:, jj * 128:(jj + 1) * 128],
                            ident_f[0:HS + 1, 0:HS + 1],
                        )
                        rec = workp.tile([128, 1], f32, name="rec", tag="rec")
                        nc.vector.reciprocal(rec, ps_on[:, HS:HS + 1])
                        nc.vector.tensor_scalar(
                            out=out4[:, jj * HS:(jj + 1) * HS], in0=ps_on[:, 0:HS],
                            scalar1=rec[:, :1], scalar2=None, op0=Alu.mult,
                        )
                    out_view = out_d[q0:q0 + QC, :].rearrange("(j p) d -> p j d", p=128)
                    nc.sync.dma_start(out=out_view,
                                      in_=out4.rearrange("p (j d) -> p j d",
                                                         j=QC // 128))

                pair_queue = []

                def emit_pairs(n):
                    for _ in range(min(n, len(pair_queue))):
                        item = pair_queue.pop(0)
                        if item[0] == "pair":
                            emit_pair(*item[1:])
                        else:
                            emit_epilogue(item[1])

                def queue_wave(qc):
                    wave_state[qc] = {"ps_o": None, "pv_pending": None}
                    ntb = _NTB_QC[qc]
                    for tb in range(0, ntb, 2):
                        pair_queue.append(("pair", qc, tb, tb + 1))
                    pair_queue.append(("epi", qc))

                gathered = set()
                flush_by_end = {b: (a, b) for a, b in _FLUSH_SEGS}

                # ---- load xT + projections, attention interleaved ----
                ps_qkv = None
                for tci in range(NTC):
                    if tci > 0 or not first:
                        xchunk(tci, nc.sync)
                    for tb in range(tci * (TCW // 128), (tci + 1) * (TCW // 128)):
                        t0 = tb * 128
                        h = (tb % 2) * 192
                        if h == 0:
                            ps_qkv = psA.tile([128, 384], f32, name="ps_qkv",
                                              tag="ps_qkv")
                        for c in range(8):
                            nc.tensor.matmul(
                                ps_qkv[:, h:h + 192], lhsT=xt3[:, c, t0:t0 + 128],
                                rhs=w_sb[c], start=(c == 0), stop=(c == 7),
                            )
                        nc.vector.tensor_tensor(
                            out=qkv3[:, tb, 0:192], in0=ps_qkv[:, h:h + 192],
                            in1=bias_bc, op=Alu.add)
                        # kT: even t-block -> partitions 0-63, odd -> 64-127
                        half = (tb % 2) * 64
                        if half == 0:
                            ps_kt = psB.tile([128, 128], bf16, name="ps_kt",
                                             tag="small")
                        nc.tensor.transpose(
                            ps_kt[half:half + 64, :], qkv3[:, tb, HS:2 * HS],
                            ident_b, tile_position=(0, half))
                        if half == 64:
                            nc.vector.tensor_copy(
                                kt2[:, (tb // 2) * 128:(tb // 2) * 128 + 128],
                                ps_kt)
                        # contiguous keep rows (t >= 3072): qgT straight from qkv
                        if tb >= _NT - _NJ // 2:
                            j = tb - (_NT - _NJ)
                            ps_q = psA.tile([128, 128], bf16, name="ps_q",
                                            tag="ps_qkv")
                            nc.tensor.transpose(
                                ps_q[0:64, :], qkv3[:, tb, 0:HS], ident_b)
                            nc.tensor.transpose(
                                ps_q[64:128, :], qkv3[:, tb, 0:HS], ident_b,
                                tile_position=(0, 64))
                            nc.vector.tensor_copy(
                                qgt2[:, j * 128:(j + 1) * 128], ps_q)
                        # q-row flush + per-j gather DMAs as prefixes complete
                        if tb + 1 in flush_by_end:
                            a, b = flush_by_end[tb + 1]
                            qv = qkv3[:, a:b, 0:HS]
                            ov = qscr[a * 128:b * 128, :].rearrange(
                                "(b p) d -> p b d", p=128)
                            nc.gpsimd.dma_start(out=ov, in_=qv)
                            for j in _GATHER_AT.get(b, ()):
                                emit_gather_dma(j)
                        # deferred gather transposes (gather DMA long done)
                        for j in _TRANSPOSE_AT.get(tb, ()):
                            emit_gather_transpose(j)
                            gathered.add(j)
                        # wave unlocks
                        for qc in range(2):
                            need_j = set(range(qc * 4, qc * 4 + 4))
                            if (qc not in wave_state
                                    and _NTB_QC[qc] <= tb + 1
                                    and need_j <= gathered):
                                queue_wave(qc)
                        for qc in range(2, NQC):
                            if qc not in wave_state and _NTB_QC[qc] <= tb + 1:
                                queue_wave(qc)
                        emit_pairs(1)
                emit_pairs(len(pair_queue))

            for _rep in range(reps):
                emit_once(_rep == 0)

    nc.compile()
    return nc


def _get_program():
    if "nc" not in _prog_cache:
        _prog_cache["nc"] = _build_program()
    return _prog_cache["nc"]


def _host_wpack(Wq, bq, Wk, bk, Wv, bv):
    wext = np.concatenate(
        [np.asarray(Wq).T, np.asarray(Wk).T, np.asarray(Wv).T], axis=1
    ).astype(np.float32)  # [C, 192]
    wpack = np.empty((128, 8 * 192), dtype=np.float32)
    for c in range(8):
        wpack[:, c * 192:(c + 1) * 192] = wext[c * 128:(c + 1) * 128, :]
    bias = np.concatenate(
        [np.asarray(bq), np.asarray(bk), np.asarray(bv)]
    ).astype(np.float32)[None, :]  # [1, 192]
    return wpack, bias


def kernel(x, Wq, bq, Wk, bk, Wv, bv):
    from concourse.bass_utils import run_bass_kernel_spmd

    x = np.asarray(x, dtype=np.float32)
    wpack, bias = _host_wpack(Wq, bq, Wk, bk, Wv, bv)
    masks = _host_masks()
    keep_u32 = np.ascontiguousarray(
        KEEP.astype(np.uint32).reshape(_NJ, 128).T)  # [128, NJ]

    nc = _get_program()
    in_maps = []
    for b in range(NCORES):
        in_maps.append({
            "xt": np.ascontiguousarray(x[b].T).astype(BF16),
            "wpack": wpack,
            "bias": bias,
            "masks": masks,
            "keepidx": keep_u32,
        })
    res = run_bass_kernel_spmd(nc, in_maps, core_ids=list(range(NCORES)),
                               trace=TRACE, **TRACE_KW)
    global LAST_RESULTS
    LAST_RESULTS = res
    out = np.stack([res.results[b]["out"] for b in range(NCORES)], axis=0)
    return out.astype(np.float32)


# revision 20
# speedup vs baseline: 1.1459x; 1.1459x over previous
"""Sparse-attention head kernel for Trainium2, data-parallel over batch on 8 cores.

Math per batch b (see reference):
  q,k,v = x @ W{q,k,v}.T + b{q,k,v}          # [T, 64]
  qg    = q[keep]                            # [K=T/2, 64]
  att   = softmax(mask(qg @ k.T / sqrt(C)))  # [K, T], row i allows t <= keep[i]
  out   = att @ v                            # [K, 64]

Device strategy (per core, one batch):
  - x[b].T uploaded bf16; first 3 t-chunks via SWDGE (Pool engine starts
    ~0.2us, HWDGE queues need ~7us to boot), rest via HWDGE, one 3D DMA
    per 512-column chunk
  - projections: per 128-t-block, qkv[t, q|k|v|1] = sum_c xT_c.T @ W_c
    packed into one persistent [128, 32*193] tile; one DVE bias-add per
    block; PSUM holds two blocks per bank to deepen the pipeline
  - kT stored two-deep (even t-blocks on partitions 0-63, odd on 64-127)
    so score matmuls for a t-block pair run as two concurrent row-group
    tiles (K=64 each) on the PE array -> ~2x ST throughput; kT copies run
    on the scalar engine, which is idle until the first exp anyway
  - qgT duplicated onto both partition halves (two PE transposes into one
    PSUM tile) to feed both row groups
  - q rows for the first two q-chunks round-trip DRAM; gathers fire per
    128-row j-block as soon as the needed t-prefix is flushed, so the
    first score matmul (and exp) start ~20us in.  The last two q-chunks
    have contiguous keep rows (t 3072..4095) and transpose straight out
    of the projection tiles
  - boundary masks applied additively (-1e30) into the score PSUM by the
    PE itself (identity-stationary accumulate), so exp depends only on PE
  - transposed attention: S_T[t,q] in PSUM, E = exp(S/sqrt(C)) on ACT,
    out_T[65,q] = sum_t [v|1].T @ E  (row 64 = softmax denominator)
  - PE-transpose out_T, divide by denominator, DMA out
All matmul inputs bf16 (fp32 accumulation in PSUM); final epilogue in fp32.
"""

import math
import os

if "JAX_PLATFORMS" not in os.environ:
    os.environ["JAX_PLATFORMS"] = "axon,cpu"

import numpy as np
import ml_dtypes

B, T, C = 8, 4096, 1024
HS = 64
KQ = T // 2  # 2048 gathered query rows
NCORES = 8
SCALE = float(C) ** -0.5
QC = 512   # attention q-chunk (matmul moving width)
BF16 = ml_dtypes.bfloat16
NQC = KQ // QC  # 4
NEG = -1.0e30
BW = HS * 3 + 1  # 193: q|k|v|1 per t-block


def _keep_indices(t):
    a = math.ceil(t / 4)
    keep = [t - 1 - x for x in range(a)]
    keep += [t - 1 - math.ceil(3 / a * (x - a) ** 2 + a) for x in range(a, math.ceil(t / 2))]
    return np.array(list(reversed(keep)), dtype=np.int64)


KEEP = _keep_indices(T)  # [KQ], ascending
# last KQ/2 keep rows are exactly t = T-KQ/2 .. T-1 (contiguous)
assert (KEEP[KQ // 2:] == np.arange(T - KQ // 2, T)).all()

# Static block classification at [t=128] x [q=128] granularity.
_NT = T // 128   # 32
_NJ = KQ // 128  # 16
_FULL, _BOUND, _DEAD = 0, 1, 2
_BLOCK_KIND = np.empty((_NT, _NJ), dtype=np.int64)
_MASK_IDX = {}
for _tb in range(_NT):
    for _j in range(_NJ):
        qlo = KEEP[_j * 128]
        qhi = KEEP[_j * 128 + 127]
        if 128 * _tb + 127 <= qlo:
            _BLOCK_KIND[_tb, _j] = _FULL
        elif 128 * _tb > qhi:
            _BLOCK_KIND[_tb, _j] = _DEAD
        else:
            _BLOCK_KIND[_tb, _j] = _BOUND
            _MASK_IDX[(_tb, _j)] = len(_MASK_IDX)
_NMASK = len(_MASK_IDX)

# t-blocks needed per q-chunk (all even -> full t-block pairs)
_NTB_QC = [int(KEEP[qc * QC + QC - 1]) // 128 + 1 for qc in range(NQC)]
assert all(n % 2 == 0 for n in _NTB_QC)

# q-scratch flush segments (t-block ranges).  Interleaving gathers between
# flushes serializes the head on DRAM round-trip latency (flush -> gather ->
# flush ...), so: two big flushes, then ALL gathers, then transposes a few
# t-blocks later so the in-order PE queue never stalls on a gather.
_FLUSH_SEGS = [(0, 12), (12, 24)]
_GATHER_AT = {24: list(range(_NJ // 2))}
_TRANSPOSE_AT = {26: [0, 1, 2, 3], 28: [4, 5, 6, 7]}


def _alive_j0(qc, tb):
    # sub-blocks j in [4qc, 4qc+4); dead ones form a prefix (keep ascending)
    for jj in range(QC // 128):
        if _BLOCK_KIND[tb, qc * (QC // 128) + jj] != _DEAD:
            return jj
    return QC // 128


def _host_masks():
    # additive masks: 0 where allowed, NEG where disallowed
    m = np.zeros((128, _NMASK * 128), dtype=np.float32)
    for (tb, j), idx in _MASK_IDX.items():
        tvals = 128 * tb + np.arange(128)[:, None]
        kvals = KEEP[j * 128:(j + 1) * 128][None, :]
        m[:, idx * 128:(idx + 1) * 128] = np.where(tvals <= kvals, 0.0, NEG)
    return m.astype(BF16)


_prog_cache = {}
TRACE = False          # set by test harness to collect an NTFF profile
TRACE_KW = {}
LAST_RESULTS = None    # BassKernelResults of the most recent kernel() call


def _build_program(reps=1):
    import concourse.bass as bass
    import concourse.mybir as mybir
    import concourse.tile as tile
    from concourse import bacc
    from concourse.masks import make_identity

    dt = mybir.dt
    f32, bf16, u32 = dt.float32, dt.bfloat16, dt.uint32
    Alu = mybir.AluOpType
    Act = mybir.ActivationFunctionType

    nc = bacc.Bacc("TRN2", target_bir_lowering=False, debug=False,
                   enable_partition_id=False)

    xt_d = nc.dram_tensor("xt", [C, T], bf16, kind="ExternalInput").ap()
    wpack_d = nc.dram_tensor("wpack", [128, 8 * 192], f32, kind="ExternalInput").ap()
    bias_d = nc.dram_tensor("bias", [1, 192], f32, kind="ExternalInput").ap()
    masks_d = nc.dram_tensor("masks", [128, _NMASK * 128], bf16, kind="ExternalInput").ap()
    keep_d = nc.dram_tensor("keepidx", [128, _NJ], u32, kind="ExternalInput").ap()
    out_d = nc.dram_tensor("out", [KQ, HS], f32, kind="ExternalOutput").ap()

    NTC = 8        # xt DMA t-chunks
    TCW = T // NTC  # 512
    NSW = 3        # leading chunks via SWDGE (Pool), rest via HWDGE (sync)
    NPAIR = _NT // 2

    with tile.TileContext(nc) as tc:
        with (
            tc.tile_pool(name="const", bufs=1) as constp,
            tc.tile_pool(name="xt", bufs=1) as xtp,
            tc.tile_pool(name="proj", bufs=1) as projp,
            tc.tile_pool(name="dram", bufs=1, space="DRAM") as dramp,
            tc.tile_pool(name="psA", bufs=2, space="PSUM") as psA,
            tc.tile_pool(name="psB", bufs=1, space="PSUM") as psB,
            tc.tile_pool(name="psS", bufs=2, space="PSUM") as psS,
            tc.tile_pool(name="psO", bufs=1, space="PSUM") as psO,
            tc.tile_pool(name="work", bufs=2) as workp,
            tc.tile_pool(name="ework", bufs=4) as ep,
        ):
            xt_big = xtp.tile([128, 8 * T], bf16, name="xt_big", tag="xt_big")
            xt3 = xt_big.rearrange("p (c t) -> p c t", c=8)
            xt_d3 = xt_d.rearrange("(c p) t -> p c t", p=128)

            def xchunk(tci, engine):
                lo, hi = tci * TCW, (tci + 1) * TCW
                engine.dma_start(out=xt3[:, :, lo:hi], in_=xt_d3[:, :, lo:hi])

            ident_b = constp.tile([128, 128], bf16)
            make_identity(nc, ident_b)
            ident_f = constp.tile([128, 128], f32)
            make_identity(nc, ident_f)
            # touch Exp so the ACT table set loads during the idle head, not
            # at the first real exp
            warm = constp.tile([128, 1], bf16)
            nc.scalar.activation(warm, ident_b[:, 0:1], Act.Exp)

            # x chunks all on one HWDGE ring, strictly ordered (mixing SWDGE
            # and HWDGE interleaves transfers on the shared DMA engines and
            # delays the first chunk's completion by several us)
            xchunk(0, nc.sync)

            wpack_sb = constp.tile([128, 8 * 192], bf16)
            nc.gpsimd.dma_start(out=wpack_sb, in_=wpack_d)
            w_sb = [wpack_sb[:, c * 192:(c + 1) * 192] for c in range(8)]
            bias_bc = constp.tile([128, 192], bf16)
            nc.gpsimd.dma_start(out=bias_bc, in_=bias_d.to_broadcast([128, 192]))

            mask_big = constp.tile([128, _NMASK * 128], bf16)
            nc.gpsimd.dma_start(out=mask_big, in_=masks_d)
            keep_big = constp.tile([128, _NJ], u32)
            nc.gpsimd.dma_start(out=keep_big, in_=keep_d)

            # ---- per-repetition kernel body (reps>1 only for timing) ----
            def emit_once(first):
                kt2 = projp.tile([128, NPAIR * 128], bf16, name="kt2", tag="kt2")
                qgt2 = projp.tile([128, KQ], bf16, name="qgt2", tag="qgt2")
                qkv = projp.tile([128, _NT * BW], bf16, name="qkv", tag="qkv")
                qkv3 = qkv.rearrange("p (b z) -> p b z", b=_NT)
                qscr = dramp.tile([T, HS], bf16, name="qscr", tag="qscr")

                # ones column of every [v|1] block, one strided memset
                nc.gpsimd.memset(qkv3[:, :, BW - 1:BW], 1.0)

                wave_state = {}

                gathered_qg = {}

                def emit_gather_dma(j):
                    # indirect row gather (j-blocks 0..7: scattered keep rows)
                    rows = 128 * (int(KEEP[j * 128 + 127]) // 128 + 1)
                    qg_g = workp.tile([128, HS], bf16, name="qg_g",
                                      tag=f"qg{j}", bufs=1)
                    nc.gpsimd.indirect_dma_start(
                        out=qg_g, out_offset=None, in_=qscr[0:rows, :],
                        in_offset=bass.IndirectOffsetOnAxis(
                            ap=keep_big[:, j:j + 1], axis=0),
                    )
                    gathered_qg[j] = qg_g

                def emit_gather_transpose(j):
                    qg_g = gathered_qg.pop(j)
                    ps_q = psA.tile([128, 128], bf16, name="ps_q", tag="ps_qkv")
                    nc.tensor.transpose(ps_q[0:64, :], qg_g, ident_b)
                    nc.tensor.transpose(ps_q[64:128, :], qg_g, ident_b,
                                        tile_position=(0, 64))
                    nc.vector.tensor_copy(qgt2[:, j * 128:(j + 1) * 128], ps_q)

                def emit_pair(qc, tba, tbb):
                    """Row-tiled ST pair + PE masks + one exp; emits previous
                    pair's PVs between the STs and the exp."""
                    st = wave_state[qc]
                    if st["ps_o"] is None:
                        st["ps_o"] = psO.tile([HS + 1, QC], f32, name=f"ps_o_{qc}",
                                              tag="ps_o")
                    q0 = qc * QC
                    i = tba // 2
                    a0a = _alive_j0(qc, tba) * 128
                    a0b = _alive_j0(qc, tbb) * 128
                    ps_s = psS.tile([128, 2 * QC], f32, name="ps_s")
                    # 4 concurrent PE tiles: row groups = t-block pair (K=64
                    # each), column groups = t-offset halves (M=64 each)
                    for rg, seg, a0_ in ((0, 0, a0a), (64, QC, a0b)):
                        for ch in (0, 64):
                            nc.tensor.matmul(
                                ps_s[ch:ch + 64, seg + a0_:seg + QC],
                                lhsT=kt2[rg:rg + 64,
                                         i * 128 + ch:i * 128 + ch + 64],
                                rhs=qgt2[rg:rg + 64, q0 + a0_:q0 + QC],
                                start=True, stop=True,
                            )
                    # additive -1e30 masks for boundary blocks, accumulated by
                    # the PE itself (identity stationary) so exp waits only on PE
                    for i2, tb in enumerate((tba, tbb)):
                        for jj in range(QC // 128):
                            j = q0 // 128 + jj
                            if _BLOCK_KIND[tb, j] == _BOUND:
                                midx = _MASK_IDX[(tb, j)]
                                o = i2 * QC + jj * 128
                                nc.tensor.matmul(
                                    ps_s[:, o:o + 128], lhsT=ident_b,
                                    rhs=mask_big[:, midx * 128:(midx + 1) * 128],
                                    start=False, stop=True, skip_group_check=True,
                                )
                    prev_pv = st["pv_pending"]
                    st["pv_pending"] = None
                    if prev_pv is not None:
                        emit_pv(qc, *prev_pv)
                    e_sb = ep.tile([128, 2 * QC], bf16, name="e_sb")
                    nc.scalar.activation(e_sb[:, a0a:2 * QC], ps_s[:, a0a:2 * QC],
                                         Act.Exp, scale=SCALE)
                    st["pv_pending"] = ((tba, tbb), e_sb, (a0a, a0b))

                def emit_pv(qc, tbs, e_sb, a0s):
                    st = wave_state[qc]
                    ntb = _NTB_QC[qc]
                    for i, tb in enumerate(tbs):
                        nc.tensor.matmul(
                            st["ps_o"][:, a0s[i]:QC],
                            lhsT=qkv3[:, tb, 2 * HS:BW],
                            rhs=e_sb[:, i * QC + a0s[i]:(i + 1) * QC],
                            start=(tb == 0), stop=(tb == ntb - 1),
                        )

                def emit_epilogue(qc):
                    st = wave_state[qc]
                    if st["pv_pending"] is not None:
                        emit_pv(qc, *st["pv_pending"])
                        st["pv_pending"] = None
                    q0 = qc * QC
                    ps_o = st["ps_o"]
                    ot_sb = workp.tile([HS + 1, QC], f32, name="ot_sb", tag="ot")
                    nc.vector.tensor_copy(ot_sb, ps_o)
                    out4 = workp.tile([128, (QC // 128) * HS], f32,
                                      name="out4", tag="out4")
                    for jj in range(QC // 128):
                        ps_on = psA.tile([128, HS + 1], f32, name="ps_on",
                                         tag="ps_qkv")
                        nc.tensor.transpose(
                            ps_on, ot_sb[:, jj * 128:(jj + 1) * 128],
                            ident_f[0:HS + 1, 0:HS + 1],
                        )
                        rec = workp.tile([128, 1], f32, name="rec", tag="rec")
                        nc.vector.reciprocal(rec, ps_on[:, HS:HS + 1])
                        nc.vector.tensor_scalar(
                            out=out4[:, jj * HS:(jj + 1) * HS], in0=ps_on[:, 0:HS],
                            scalar1=rec[:, :1], scalar2=None, op0=Alu.mult,
                        )
                    out_view = out_d[q0:q0 + QC, :].rearrange("(j p) d -> p j d", p=128)
                    nc.sync.dma_start(out=out_view,
                                      in_=out4.rearrange("p (j d) -> p j d",
                                                         j=QC // 128))

                pair_queue = []

                def emit_pairs(n):
                    for _ in range(min(n, len(pair_queue))):
                        item = pair_queue.pop(0)
                        if item[0] == "pair":
                            emit_pair(*item[1:])
                        else:
                            emit_epilogue(item[1])

                def queue_wave(qc):
                    wave_state[qc] = {"ps_o": None, "pv_pending": None}
                    ntb = _NTB_QC[qc]
                    for tb in range(0, ntb, 2):
                        pair_queue.append(("pair", qc, tb, tb + 1))
                    pair_queue.append(("epi", qc))

                gathered = set()
                flush_by_end = {b: (a, b) for a, b in _FLUSH_SEGS}

                # ---- load xT + projections, attention interleaved ----
                ps_qkv = None
                for tci in range(NTC):
                    if tci > 0 or not first:
                        xchunk(tci, nc.sync)
                    for tb in range(tci * (TCW // 128), (tci + 1) * (TCW // 128)):
                        t0 = tb * 128
                        h = (tb % 2) * 192
                        if h == 0:
                            ps_qkv = psA.tile([128, 384], f32, name="ps_qkv",
                                              tag="ps_qkv")
                        for c in range(8):
                            nc.tensor.matmul(
                                ps_qkv[:, h:h + 192], lhsT=xt3[:, c, t0:t0 + 128],
                                rhs=w_sb[c], start=(c == 0), stop=(c == 7),
                            )
                        nc.vector.tensor_tensor(
                            out=qkv3[:, tb, 0:192], in0=ps_qkv[:, h:h + 192],
                            in1=bias_bc, op=Alu.add)
                        # kT: even t-block -> partitions 0-63, odd -> 64-127
                        half = (tb % 2) * 64
                        if half == 0:
                            ps_kt = psB.tile([128, 128], bf16, name="ps_kt",
                                             tag="small")
                        nc.tensor.transpose(
                            ps_kt[half:half + 64, :], qkv3[:, tb, HS:2 * HS],
                            ident_b, tile_position=(0, half))
                        if half == 64:
                            nc.vector.tensor_copy(
                                kt2[:, (tb // 2) * 128:(tb // 2) * 128 + 128],
                                ps_kt)
                        # contiguous keep rows (t >= 3072): qgT straight from qkv
                        if tb >= _NT - _NJ // 2:
                            j = tb - (_NT - _NJ)
                            ps_q = psA.tile([128, 128], bf16, name="ps_q",
                                            tag="ps_qkv")
                            nc.tensor.transpose(
                                ps_q[0:64, :], qkv3[:, tb, 0:HS], ident_b)
                            nc.tensor.transpose(
                                ps_q[64:128, :], qkv3[:, tb, 0:HS], ident_b,
                                tile_position=(0, 64))
                            nc.vector.tensor_copy(
                                qgt2[:, j * 128:(j + 1) * 128], ps_q)
                        # q-row flush + per-j gather DMAs as prefixes complete
                        if tb + 1 in flush_by_end:
                            a, b = flush_by_end[tb + 1]
                            qv = qkv3[:, a:b, 0:HS]
                            ov = qscr[a * 128:b * 128, :].rearrange(
                                "(b p) d -> p b d", p=128)
                            nc.gpsimd.dma_start(out=ov, in_=qv)
                            for j in _GATHER_AT.get(b, ()):
                                emit_gather_dma(j)
                        # deferred gather transposes (gather DMA long done)
                        for j in _TRANSPOSE_AT.get(tb, ()):
                            emit_gather_transpose(j)
                            gathered.add(j)
                        # wave unlocks
                        for qc in range(2):
                            need_j = set(range(qc * 4, qc * 4 + 4))
                            if (qc not in wave_state
                                    and _NTB_QC[qc] <= tb + 1
                                    and need_j <= gathered):
                                queue_wave(qc)
                        for qc in range(2, NQC):
                            if qc not in wave_state and _NTB_QC[qc] <= tb + 1:
                                queue_wave(qc)
                        emit_pairs(1)
                emit_pairs(len(pair_queue))

            for _rep in range(reps):
                emit_once(_rep == 0)

    nc.compile()
    return nc


def _get_program():
    if "nc" not in _prog_cache:
        _prog_cache["nc"] = _build_program()
    return _prog_cache["nc"]


def _host_wpack(Wq, bq, Wk, bk, Wv, bv):
    wext = np.concatenate(
        [np.asarray(Wq).T, np.asarray(Wk).T, np.asarray(Wv).T], axis=1
    ).astype(np.float32)  # [C, 192]
    wpack = np.empty((128, 8 * 192), dtype=np.float32)
    for c in range(8):
        wpack[:, c * 192:(c + 1) * 192] = wext[c * 128:(c + 1) * 128, :]
    bias = np.concatenate(
        [np.asarray(bq), np.asarray(bk), np.asarray(bv)]
    ).astype(np.float32)[None, :]  # [1, 192]
    return wpack, bias


def kernel(x, Wq, bq, Wk, bk, Wv, bv):
    from concourse.bass_utils import run_bass_kernel_spmd

    x = np.asarray(x, dtype=np.float32)
    wpack, bias = _host_wpack(Wq, bq, Wk, bk, Wv, bv)
    masks = _host_masks()
    keep_u32 = np.ascontiguousarray(
        KEEP.astype(np.uint32).reshape(_NJ, 128).T)  # [128, NJ]

    nc = _get_program()
    in_maps = []
    for b in range(NCORES):
        in_maps.append({
            "xt": np.ascontiguousarray(x[b].T).astype(BF16),
            "wpack": wpack,
            "bias": bias,
            "masks": masks,
            "keepidx": keep_u32,
        })
    res = run_bass_kernel_spmd(nc, in_maps, core_ids=list(range(NCORES)),
                               trace=TRACE, **TRACE_KW)
    global LAST_RESULTS
    LAST_RESULTS = res
    out = np.stack([res.results[b]["out"] for b in range(NCORES)], axis=0)
    return out.astype(np.float32)
